# revision 18
# baseline (speedup 1.0000x reference)
"""Trainium2 Bass kernel for nn_ChiralEmbeddingModel (chiral tensor-product embedding).

v2: bf16 datapath with deferred normalization.

Math (per atom n, with x = atomic_embeddings[n, 256:].reshape(128, 3)):
    ms   = mean(x^2)                 (over all 384 components)
    s3   = (ms + eps)^-1.5           (rms normalization, cubic in x -> fold at output)
    y    = w1' @ x^T                 (w1'[u,v] = C1 * g[v] * w1[u,v], unnormalized x)
    cr_i = eps_ijk x_j y_k           (cross product per mul-channel)
    z    = w2' @ cr                  (w2'[u,v] = C2 * g[v] * w2[u,v])
    chi  = sum_i x_i * z_i
    out  = s3 * (chi @ Wo') + b      (Wo'[u,o] = g[u] * W_out[o,u])

Everything on-device runs in bf16 (matmuls 1 cyc/row, DVE tensor_tensor at
the 2x perf mode) except PSUM accumulation, which is fp32.  The per-atom
1/rms^3 factor is applied by the output eviction (scalar-engine activation
with a per-partition scale), where atoms sit on partitions; the bias is
added on the host during the unshard (it is a plain broadcast add).  Work
is spread across the engines per 512-atom tile:
    PE    : 12 transposes + 3 y + 6 z + 4 out matmuls
    DVE   : squared-norm accumulation, 6 cross products, 3 dot products,
            2 chi adds
    Act   : sqrt, xt/y evictions (batched [128,1536]), out evictions with
            the fused s3 scale
The z matmuls read their products from PSUM-resident z via the DVE (1x)
rather than paying a separate z eviction.  PSUM: two rotating 3-bank
[128,3,512] slots (xt -> y -> z) + two 1-bank out slots = exactly 8 banks.
Sharding: pure data-parallel over the atom axis across 8 NeuronCores.
"""

import numpy as np
import ml_dtypes

N_TOTAL = 131072
N_CORES = 8
N_SHARD = N_TOTAL // N_CORES  # 16384
INV = 256
MUL = 128
EDIM = 3
F = MUL * EDIM  # 384
OUT = 512
EPS = 1e-6
C1 = (3.0 / 256.0) ** 0.5
C2 = (1.0 / 384.0) ** 0.5
P = 128
TILE_ATOMS = 512
NCHUNK = TILE_ATOMS // P  # 4

# cross product index pairs: cr_0 = x1*y2 - x2*y1, etc.
PLUS = [(1, 2), (2, 0), (0, 1)]
MINUS = [(2, 1), (0, 2), (1, 0)]


def _build_nc(
    n_shard: int,
    repeat: int = 1,
    loop_repeat: int = 1,
    ms_engine: str = "vector",   # vector | scalar | gpsimd | mix (2 DVE + 2 Pool)
    xt_evict: str = "scalar",    # scalar | vector
    y_evict: str = "scalar",     # scalar | vector
    z_evict: str = "none",       # none (cprod reads PSUM) | scalar | vector
    out_evict: str = "scalar",   # scalar | vector | mix3 (3 Act + 1 DVE)
    chi_engine: str = "vector",  # gpsimd | vector
    pool_bprods: int = 0,        # 0..6 bprods on the gpsimd engine
    bias: str = "host",          # pe | host
    rsqrt: str = "sqrt_recip",   # sqrt_recip (Act.Rsqrt is blocked by bass)
    mm_dtype: str = "bf16",      # kept for CLI compat; bf16 only path
    skew: int = 0,               # software-pipeline skew (tiles of head lead)
    psum_mode: str = "batch",    # batch ([P,3,T] tiles) | comp (per-component)
    tile_atoms: int = TILE_ATOMS,
    ps3_bufs: int = None,        # default: 2 for 512-atom tiles, 3 for 256
    sb_bufs: int = 4,            # buffer depth of SBUF stage pools
):
    import concourse.bass as bass
    import concourse.bacc as bacc
    import concourse.tile as tile
    from concourse import mybir

    f32 = mybir.dt.float32
    bf16 = mybir.dt.bfloat16
    Alu = mybir.AluOpType
    Act = mybir.ActivationFunctionType

    TILE = tile_atoms
    NCH = TILE // P
    if ps3_bufs is None:
        ps3_bufs = 2 if TILE >= 512 else 3
    assert n_shard % TILE == 0
    n_tiles = n_shard // TILE

    nc = bacc.Bacc("TRN2", target_bir_lowering=False, debug=False)

    # Register EPS as a const AP so activation bias=EPS lowers to a
    # dependency-free const read.
    _eps_t = nc.alloc_sbuf_tensor("const-float32-eps", [128, 1], f32)
    nc.gpsimd.memset(_eps_t.ap(), EPS)
    nc.const_aps.aps[(f32, EPS)] = _eps_t.ap()
    nc.all_engine_barrier()

    xs = nc.dram_tensor("xs", [n_shard, F], bf16, kind="ExternalInput").ap()
    w1t = nc.dram_tensor("w1t", [MUL, MUL], bf16, kind="ExternalInput").ap()
    w2pt = nc.dram_tensor("w2pt", [MUL, MUL], bf16, kind="ExternalInput").ap()
    w2mt = nc.dram_tensor("w2mt", [MUL, MUL], bf16, kind="ExternalInput").ap()
    wot = nc.dram_tensor("wot", [MUL, OUT], bf16, kind="ExternalInput").ap()
    ident = nc.dram_tensor("ident", [P, P], bf16, kind="ExternalInput").ap()
    brow = nc.dram_tensor("brow", [1, OUT], bf16, kind="ExternalInput").ap()
    onesrow = nc.dram_tensor("onesrow", [1, P], bf16, kind="ExternalInput").ap()
    out = nc.dram_tensor("out", [n_shard, OUT], bf16, kind="ExternalOutput").ap()

    with tile.TileContext(nc) as tc:
        with (
            tc.tile_pool(name="singles", bufs=1) as singles,
            tc.tile_pool(name="xin", bufs=3) as xin_pool,
            tc.tile_pool(name="stats", bufs=3) as stats_pool,
            tc.tile_pool(name="sq", bufs=2) as sq_pool,
            tc.tile_pool(name="xt", bufs=sb_bufs) as xt_pool,
            tc.tile_pool(name="ysb", bufs=sb_bufs) as y_pool,
            tc.tile_pool(name="zsb", bufs=sb_bufs) as z_pool,
            tc.tile_pool(name="bprod", bufs=sb_bufs) as b_pool,
            tc.tile_pool(name="cprod", bufs=sb_bufs) as c_pool,
            tc.tile_pool(name="chi", bufs=2 * sb_bufs) as chi_pool,
            tc.tile_pool(name="outs", bufs=sb_bufs) as out_pool,
            tc.tile_pool(name="psum", bufs=2, space="PSUM") as psum_pool,
        ):
            # ---- load replicated constants once ----
            w1t_sb = singles.tile([MUL, MUL], bf16)
            w2pt_sb = singles.tile([MUL, MUL], bf16)
            w2mt_sb = singles.tile([MUL, MUL], bf16)
            wot_sb = singles.tile([MUL, OUT], bf16)
            ident_sb = singles.tile([P, P], bf16)
            brow_sb = singles.tile([1, OUT], bf16)
            ones_sb = singles.tile([1, P], bf16)
            nc.sync.dma_start(out=w1t_sb, in_=w1t)
            nc.sync.dma_start(out=w2pt_sb, in_=w2pt)
            nc.sync.dma_start(out=w2mt_sb, in_=w2mt)
            nc.sync.dma_start(out=wot_sb, in_=wot)
            nc.sync.dma_start(out=ident_sb, in_=ident)
            nc.sync.dma_start(out=brow_sb, in_=brow)
            nc.sync.dma_start(out=ones_sb, in_=onesrow)

            xs_t = xs.rearrange("(t c p) f -> t c p f", c=NCH, p=P)
            out_t = out.rearrange("(t c p) o -> t c p o", c=NCH, p=P)

            import contextlib

            loop_cm = (
                tc.For_i(0, loop_repeat, 1)
                if loop_repeat > 1
                else contextlib.nullcontext()
            )

            def head(it):
                # ---- load: [128, 4, 384] bf16
                x_in = xin_pool.tile([P, NCH, F], bf16, tag="x_in")
                nc.sync.dma_start(
                    out=x_in, in_=xs_t[it].rearrange("c p f -> p c f")
                )
                x_uj = x_in.rearrange("p c (u j) -> p c u j", j=EDIM)

                # ---- per-atom sum of squares -> stats[:, c]
                stats = stats_pool.tile([P, NCH], f32, tag="stats")
                for c in range(NCH):
                    sq_junk = sq_pool.tile([P, F], bf16, tag="sq")
                    eng = ms_engine
                    if eng == "mix":
                        eng = "gpsimd" if c % 2 == 0 else "vector"
                    if eng == "scalar":
                        nc.scalar.activation(
                            out=sq_junk, in_=x_in[:, c], func=Act.Square,
                            accum_out=stats[:, c : c + 1],
                        )
                    elif eng == "gpsimd":
                        nc.gpsimd.scalar_tensor_tensor(
                            out=sq_junk, in0=x_in[:, c], scalar=1.0,
                            in1=x_in[:, c], op0=Alu.mult, op1=Alu.mult,
                            accum_out=stats[:, c : c + 1],
                        )
                    else:
                        nc.vector.scalar_tensor_tensor(
                            out=sq_junk, in0=x_in[:, c], scalar=1.0,
                            in1=x_in[:, c], op0=Alu.mult, op1=Alu.mult,
                            accum_out=stats[:, c : c + 1],
                        )

                # ---- s3 = (ms + eps)^-1.5 per atom
                if rsqrt == "act":
                    s1 = stats_pool.tile([P, NCH], f32, tag="s1")
                    nc.scalar.activation(
                        out=s1, in_=stats, func=Act.Rsqrt, scale=1.0 / F,
                        bias=EPS,
                    )
                else:
                    snorm = stats_pool.tile([P, NCH], f32, tag="snorm")
                    nc.scalar.activation(
                        out=snorm, in_=stats, func=Act.Sqrt, scale=1.0 / F,
                        bias=EPS,
                    )
                    s1 = stats_pool.tile([P, NCH], f32, tag="s1")
                    nc.vector.reciprocal(out=s1, in_=snorm)
                s2 = stats_pool.tile([P, NCH], f32, tag="s2")
                nc.vector.tensor_mul(s2, s1, s1)
                s3 = stats_pool.tile([P, NCH], f32, tag="s3")
                nc.vector.tensor_mul(s3, s2, s1)

                # ---- transposes: xt[u, j, a] for the 512-atom tile (PSUM)
                nps = 8 if psum_mode == "flat" else 6
                if psum_mode == "batch":
                    xt_ps = psum_pool.tile(
                        [P, EDIM, TILE], f32, tag="ps3", bufs=ps3_bufs, name="xt_ps"
                    )
                    xt_parts = [xt_ps[:, j] for j in range(EDIM)]
                else:
                    xt_tiles = [
                        psum_pool.tile(
                            [P, TILE], f32, tag="ps", bufs=nps,
                            name=f"xt_ps{j}",
                        )
                        for j in range(EDIM)
                    ]
                    xt_parts = xt_tiles
                for j in range(EDIM):
                    for c in range(NCH):
                        nc.tensor.matmul(
                            xt_parts[j][:, c * P : (c + 1) * P],
                            x_uj[:, c, :, j],
                            ident_sb,
                            start=True,
                            stop=True,
                        )
                xt_sb = xt_pool.tile([P, EDIM, TILE], bf16, tag="xt")

                def ev(out_ap, in_ap, which):
                    if which == "scalar":
                        nc.scalar.copy(out_ap, in_ap)
                    else:
                        nc.vector.tensor_copy(out_ap, in_ap)

                if psum_mode == "batch":
                    ev(xt_sb, xt_ps, xt_evict)
                else:
                    for j in range(EDIM):
                        ev(xt_sb[:, j], xt_parts[j], xt_evict)
                return dict(it=it, xt_sb=xt_sb, s3=s3, ev=ev)

            def midtail(st):
                it, xt_sb, s3, ev = st["it"], st["xt_sb"], st["s3"], st["ev"]
                # ---- y_k = w1' @ x_k   (PSUM -> SBUF bf16)
                if psum_mode == "batch":
                    y_ps = psum_pool.tile(
                        [P, EDIM, TILE], f32, tag="ps3", bufs=ps3_bufs, name="y_ps"
                    )
                    y_parts = [y_ps[:, k] for k in range(EDIM)]
                else:
                    nps = 8 if psum_mode == "flat" else 6
                    y_parts = [
                        psum_pool.tile(
                            [P, TILE], f32, tag="ps", bufs=nps,
                            name=f"y_ps{k}",
                        )
                        for k in range(EDIM)
                    ]
                for k in range(EDIM):
                    nc.tensor.matmul(
                        y_parts[k], w1t_sb, xt_sb[:, k],
                        start=True, stop=True,
                    )
                y_sb = y_pool.tile([P, EDIM, TILE], bf16, tag="ysb")
                if psum_mode == "batch":
                    ev(y_sb, y_ps, y_evict)
                else:
                    for k in range(EDIM):
                        ev(y_sb[:, k], y_parts[k], y_evict)

                # ---- B products for the cross terms (bf16, SBUF)
                # slot layout: [p0 p1 p2 m0 m1 m2]
                bp = b_pool.tile([P, 2 * EDIM, TILE], bf16, tag="bp")
                prods = [(i, a, b) for i, (a, b) in enumerate(PLUS)] + [
                    (3 + i, a, b) for i, (a, b) in enumerate(MINUS)
                ]
                for slot, (idx, a, b) in enumerate(prods):
                    tgt = bp[:, prods[slot][0]]
                    if slot < pool_bprods:
                        nc.gpsimd.tensor_mul(tgt, xt_sb[:, a], y_sb[:, b])
                    else:
                        nc.vector.tensor_mul(tgt, xt_sb[:, a], y_sb[:, b])

                # ---- z_i = w2p' @ B_plus[i] + w2m' @ B_minus[i]  (PSUM accum)
                if psum_mode == "batch":
                    z_ps = psum_pool.tile(
                        [P, EDIM, TILE], f32, tag="ps3", bufs=ps3_bufs, name="z_ps"
                    )
                    z_parts = [z_ps[:, i] for i in range(EDIM)]
                else:
                    nps = 8 if psum_mode == "flat" else 6
                    z_parts = [
                        psum_pool.tile(
                            [P, TILE], f32, tag="ps", bufs=nps,
                            name=f"z_ps{i}",
                        )
                        for i in range(EDIM)
                    ]
                for i in range(EDIM):
                    nc.tensor.matmul(
                        z_parts[i], w2pt_sb, bp[:, i],
                        start=True, stop=False,
                    )
                for i in range(EDIM):
                    nc.tensor.matmul(
                        z_parts[i], w2mt_sb, bp[:, 3 + i],
                        start=False, stop=True,
                    )

                # ---- chi = sum_i x_i * z_i
                if z_evict == "none":
                    z_srcs = z_parts
                else:
                    z_sb = z_pool.tile([P, EDIM, TILE], bf16, tag="zsb")
                    if psum_mode == "batch":
                        ev(z_sb, z_ps, z_evict)
                    else:
                        for i in range(EDIM):
                            ev(z_sb[:, i], z_parts[i], z_evict)
                    z_srcs = [z_sb[:, i] for i in range(EDIM)]
                cp = c_pool.tile([P, EDIM, TILE], bf16, tag="cp")
                for i in range(EDIM):
                    nc.vector.tensor_mul(cp[:, i], xt_sb[:, i], z_srcs[i])
                chi01 = chi_pool.tile([P, TILE], bf16, tag="chi")
                chi = chi_pool.tile([P, TILE], bf16, tag="chi")
                if chi_engine == "gpsimd":
                    nc.gpsimd.tensor_add(chi01, cp[:, 0], cp[:, 1])
                    nc.gpsimd.tensor_add(chi, chi01, cp[:, 2])
                else:
                    nc.vector.tensor_add(chi01, cp[:, 0], cp[:, 1])
                    nc.vector.tensor_add(chi, chi01, cp[:, 2])

                # ---- out chunks: psum = chi_chunk.T @ Wo' (bf16 operands)
                out_sb = out_pool.tile([P, NCH, OUT], bf16, tag="osb")
                for c in range(NCH):
                    if psum_mode == "flat":
                        o_ps = psum_pool.tile(
                            [P, OUT], f32, tag="ps", bufs=8, name=f"o_ps{c}"
                        )
                    else:
                        o_ps = psum_pool.tile([P, OUT], f32, tag="pso")
                    if bias == "pe":
                        nc.tensor.matmul(
                            o_ps, ones_sb, brow_sb,
                            start=True, stop=False,
                        )
                    nc.tensor.matmul(
                        o_ps,
                        chi[:, c * P : (c + 1) * P],
                        wot_sb,
                        start=(bias != "pe"),
                        stop=True,
                    )
                    # eviction applies the deferred rms scale s3 (per atom,
                    # atoms on partitions here)
                    n_act = {"scalar": 4, "vector": 0, "mix1": 1,
                             "mix2": 2, "mix3": 3}[out_evict]
                    use_scalar = c < n_act
                    if use_scalar:
                        nc.scalar.activation(
                            out=out_sb[:, c], in_=o_ps, func=Act.Copy,
                            scale=s3[:, c : c + 1],
                        )
                    else:
                        nc.vector.tensor_scalar_mul(
                            out_sb[:, c], o_ps, s3[:, c : c + 1]
                        )
                nc.sync.dma_start(
                    out=out_t[it].rearrange("c p o -> p c o"), in_=out_sb
                )

            with loop_cm:
                for _rep in range(repeat):
                    pending = []
                    for i in range(n_tiles + skew):
                        if i < n_tiles:
                            pending.append(head(i))
                        if i >= skew:
                            midtail(pending.pop(0))

    nc.finalize()
    return nc


def _host_prep(inputs, bias="pe"):
    bf = ml_dtypes.bfloat16
    emb = np.asarray(inputs["atomic_embeddings"], dtype=np.float32)
    g = np.asarray(inputs["rms_g"], dtype=np.float32)
    w1 = np.asarray(inputs["w1"], dtype=np.float32)
    w2 = np.asarray(inputs["w2"], dtype=np.float32)
    W_out = np.asarray(inputs["W_out"], dtype=np.float32)
    b_out = np.asarray(inputs["b_out"], dtype=np.float32)

    xs_full = np.ascontiguousarray(emb[:, INV:]).astype(bf)  # [N, 384] bf16
    consts = {
        "w1t": np.ascontiguousarray(C1 * (w1.T * g[:, None])).astype(bf),
        "w2pt": np.ascontiguousarray(C2 * (w2.T * g[:, None])).astype(bf),
        "w2mt": np.ascontiguousarray(-C2 * (w2.T * g[:, None])).astype(bf),
        "wot": np.ascontiguousarray(W_out.T * g[:, None]).astype(bf),
        "ident": np.eye(P, dtype=np.float32).astype(bf),
        "brow": b_out.reshape(1, OUT).astype(bf),
        "onesrow": np.ones((1, P), dtype=np.float32).astype(bf),
    }
    return xs_full, consts


_NC_CACHE = {}


def _get_nc(n_shard, **kwargs):
    key = (n_shard, tuple(sorted(kwargs.items())))
    if key not in _NC_CACHE:
        _NC_CACHE[key] = _build_nc(n_shard, **kwargs)
    return _NC_CACHE[key]


BIAS_MODE = "host"


def kernel(**inputs) -> np.ndarray:
    from concourse.bass_utils import run_bass_kernel_spmd

    xs_full, consts = _host_prep(inputs, bias=BIAS_MODE)
    n = xs_full.shape[0]
    assert n == N_TOTAL, f"expected {N_TOTAL} atoms, got {n}"

    nc = _get_nc(N_SHARD, bias=BIAS_MODE)
    in_maps = []
    for i in range(N_CORES):
        m = {"xs": xs_full[i * N_SHARD : (i + 1) * N_SHARD]}
        m.update(consts)
        in_maps.append(m)

    res = run_bass_kernel_spmd(nc, in_maps, list(range(N_CORES)))
    full = np.concatenate(
        [np.asarray(res.results[i]["out"]) for i in range(N_CORES)], axis=0
    ).astype(np.float32)
    if BIAS_MODE == "host":
        full += np.asarray(inputs["b_out"], dtype=np.float32)[None, :]
    return full
